# revision 1
# baseline (speedup 1.0000x reference)
"""Trainium2 Bass kernel for nn_ActionNetwork (gnn_message_passing).

Strategy (pure data parallel over the episode axis, 8 cores):
  - Hybrid precision: fp16 front-end, fp32 tail.  One fp16 input stream
    (x + ones column) replaces the baseline's fp32 + bf16-hi/lo triple
    (11.7 MB -> 2.9 MB per core); the 2e-2 tolerance gives fp16 input
    rounding ~30x of margin.
  - Host folds the whole linear front-end (per-node W0 gather + W1 mix +
    pairwise potential difference + distribute_param scale + bias) into
    one (89 -> 64) fp16 block, and queue*queue_param into a second
    one-hot block; both ride ONE matmul per 128 episodes (moving operand
    [89, 128] = [dmat | qmat]).  Per 128 episodes the PE now runs
    1 transpose + 1 matmul instead of the baseline's 2 + 6, cutting the
    PE sequencer stream (the baseline's critical path) ~4x.
  - val = relu(diff) + qq is assembled per chunk: ACT relu PSUM->SBUF,
    then a DVE tensor_add with one PSUM operand (the baseline's exact
    pattern).
  - The val/action/price tail is the baseline's fp32 pipeline op-for-op
    (same engines), reading veh/queue/mini straight from the fp16 xin.
"""

import numpy as np

import concourse.bass as bass
import concourse.tile as tile
from concourse import bacc, mybir
from concourse.bass_utils import run_bass_kernel_spmd

F32 = mybir.dt.float32
FP16 = mybir.dt.float16
ALU = mybir.AluOpType
ACTF = mybir.ActivationFunctionType

N = 8
MINI = 2
EP = 131072
F = 88
FI = 89
NCORES = 8
EPC = EP // NCORES          # 16384 episodes per core
BLK = 16                    # 128-episode blocks per btile
BT = 128 * BLK              # 2048 episodes per btile
NBT = EPC // BT             # 8 btiles per core

_CACHE = {}


def _kernel_body(tc, outc, xf, dmat_d, iden_d, bench_reps=None, stages='full',
                 passes=1):
    nc = tc.nc
    from contextlib import ExitStack
    ctx = ExitStack()
    with ctx:
        const_pool = ctx.enter_context(tc.tile_pool(name="const", bufs=1))
        xin_pool = ctx.enter_context(tc.tile_pool(name="xin", bufs=3))
        xt_pool = ctx.enter_context(tc.tile_pool(name="xt", bufs=6))
        pair_pool = ctx.enter_context(tc.tile_pool(name="pair", bufs=3))
        tail_pool = ctx.enter_context(tc.tile_pool(name="tail", bufs=3))
        node_pool = ctx.enter_context(tc.tile_pool(name="node", bufs=3))
        stag_pool = ctx.enter_context(tc.tile_pool(name="stag", bufs=3))
        ps_xt = ctx.enter_context(tc.tile_pool(name="ps_xt", bufs=2, space="PSUM"))
        ps_dn = ctx.enter_context(tc.tile_pool(name="ps_dn", bufs=3, space="PSUM"))
        ps_qq = ctx.enter_context(tc.tile_pool(name="ps_qq", bufs=1, space="PSUM"))

        # constants on the ACT HWDGE ring so they don't queue behind the
        # episode loads
        iden_t = const_pool.tile([128, 128], FP16, tag="iden")
        nc.scalar.dma_start(iden_t[:], iden_d)
        dm_t = const_pool.tile([FI, 128], FP16, tag="dmat")
        nc.scalar.dma_start(dm_t[:], dmat_d)
        c04_t = const_pool.tile([128, 1], F32, tag="c04")
        nc.vector.memset(c04_t[:], 0.4)

        # episode -> (partition, chunk) mapping: partition p owns the 16
        # consecutive episodes [2048*bb + 16p, +16); chunk k picks the k-th.
        # Every DMA runs 16 rows contiguous in DRAM; the whole pipeline is
        # per-episode so the permutation is harmless.
        xf_r = xf.rearrange("(bb p k) f -> bb p k f", p=128, k=BLK)
        out_r = outc.rearrange("(bb p k) o -> bb p k o", p=128, k=BLK)

        def bc(node_ap3):
            # (128, BLK, N) node tensor -> broadcast over trailing pair dim
            return node_ap3.unsqueeze(3).broadcast_to((128, BLK, N, N))

        if bench_reps is not None:
            loop_cm = tc.For_i(
                0, bench_reps, 1,
                hint_engines=(mybir.EngineType.PE, mybir.EngineType.DVE,
                              mybir.EngineType.Activation),
            )
            ctx.enter_context(loop_cm)

        for b in range(NBT * passes):
            b = b % NBT
            xin = xin_pool.tile([128, BLK * FI], FP16, tag="xin")
            xin3 = xin[:].rearrange("p (k f) -> p k f", f=FI)
            if b == 0:
                # fine-grained first load so the PE front-end starts early
                for c in range(BLK // 4):
                    nc.sync.dma_start(
                        xin3[:, 4 * c:4 * c + 4, :], xf_r[b][:, 4 * c:4 * c + 4, :]
                    )
            else:
                nc.sync.dma_start(xin3, xf_r[b])

            # transpose 128-episode chunks, then one [89->128] matmul per
            # chunk against [dmat | qmat]; val = relu(diff) + qq assembled
            # per chunk (ACT relu, DVE add with one PSUM operand)
            va = pair_pool.tile([128, BLK * 64], F32, tag="va")
            qqn = ps_qq.tile([128, BLK * 64], F32, tag="qqn")
            for c in range(BLK // 4):
                xtA = ps_xt.tile([FI, 512], FP16, tag="xtA")
                for kk in range(4):
                    k = 4 * c + kk
                    nc.tensor.transpose(
                        xtA[:, 128 * kk:128 * kk + 128], xin3[:, k, :],
                        iden_t[:],
                    )
                xt_c = xt_pool.tile([FI, 512], FP16, tag="xt")
                nc.scalar.copy(xt_c[:], xtA[:])
                diffnat = ps_dn.tile([128, 256], F32, tag="diffnat")
                for kk in range(4):
                    k = 4 * c + kk
                    xck = xt_c[:, 128 * kk:128 * kk + 128]
                    nc.tensor.matmul(diffnat[:, 64 * kk:64 * kk + 64],
                                     xck, dm_t[:, 0:64],
                                     start=True, stop=True)
                    nc.tensor.matmul(qqn[:, 64 * k:64 * k + 64],
                                     xck, dm_t[:, 64:128],
                                     start=True, stop=True)
                # va = relu(diff), psum -> sbuf per c-chunk (contiguous)
                nc.scalar.activation(
                    va[:, 256 * c:256 * c + 256], diffnat[:], ACTF.Relu
                )

            # val = relu(diff) + qq, one PSUM operand (baseline pattern)
            val = pair_pool.tile([128, BLK * 64], F32, tag="val")
            val3 = val[:].rearrange("p (k d) -> p k d", d=64)
            val4 = val[:].rearrange("p (k a b) -> p k a b", a=N, b=N)
            nc.vector.tensor_add(val[:], va[:], qqn[:])

            if stages == 'front':
                nc.sync.dma_start(out_r[b][:, :, 0:64], val3)
                continue

            # ---- natural-layout pair/node pipeline (fp32, as baseline) ----
            # stage xin to fp32 once (mixed-dtype vector ops are slow on HW)
            xin32 = xin_pool.tile([128, BLK * FI], F32, tag="xin32")
            nc.scalar.copy(xin32[:], xin[:])
            xin3f = xin32[:].rearrange("p (k f) -> p k f", f=FI)
            queue4 = xin3f[:, :, 24:88].rearrange("p k (a b) -> p k a b", b=N)
            veh3 = xin3f[:, :, 0:8]

            rs = node_pool.tile([128, BLK * N], F32, tag="rs")
            rs3 = rs[:].rearrange("p (k i) -> p k i", i=N)
            nc.vector.tensor_reduce(rs3, val4, axis=mybir.AxisListType.X, op=ALU.add)

            # denom = relu(veh - rs) + rs == max(veh, rs)
            denom = node_pool.tile([128, BLK * N], F32, tag="denom")
            denom3 = denom[:].rearrange("p (k i) -> p k i", i=N)
            nc.vector.tensor_tensor(denom3, veh3, rs3, op=ALU.max)
            rden = node_pool.tile([128, BLK * N], F32, tag="rden")
            rden3 = rden[:].rearrange("p (k i) -> p k i", i=N)
            nc.vector.reciprocal(rden[:], denom[:])

            rv = node_pool.tile([128, BLK * N], F32, tag="rv")
            rv3 = rv[:].rearrange("p (k i) -> p k i", i=N)
            nc.vector.tensor_mul(rv3, veh3, rden3)

            s_t = node_pool.tile([128, BLK * N], F32, tag="s_t")
            s3 = s_t[:].rearrange("p (k i) -> p k i", i=N)
            nc.scalar.activation(s_t[:], rv[:], ACTF.Relu, bias=1.0, scale=-1.0)

            # t_diag = relu(remain)*rden = 1 - rs*rden
            g_t = node_pool.tile([128, BLK * N], F32, tag="g_t")
            nc.vector.tensor_mul(g_t[:], rs[:], rden[:])
            t_diag = node_pool.tile([128, BLK * N], F32, tag="t_diag")
            t_diag3 = t_diag[:].rearrange("p (k i) -> p k i", i=N)
            nc.scalar.activation(t_diag[:], g_t[:], ACTF.Copy,
                                 bias=1.0, scale=-1.0)

            dep = node_pool.tile([128, BLK * N], F32, tag="dep")
            nc.gpsimd.tensor_mul(dep[:], rv[:], rs[:])

            m_t = node_pool.tile([128, BLK * N], F32, tag="m_t")
            m3 = m_t[:].rearrange("p (k i) -> p k i", i=N)
            nc.gpsimd.tensor_add(m3, xin3f[:, :, 8:24:2], xin3f[:, :, 9:24:2])

            raw = pair_pool.tile([128, BLK * 64], F32, tag="raw")
            raw4 = raw[:].rearrange("p (k a b) -> p k a b", a=N, b=N)
            nc.vector.tensor_mul(raw4, val4, bc(rv3))

            stag = stag_pool.tile([128, BLK * 128], FP16, tag="stag")
            stag5 = stag[:].rearrange("p (k i c) -> p k i c", i=N, c=2 * N)
            stag3 = stag[:].rearrange("p (k d) -> p k d", d=2 * N * N)
            # action off-diagonal (diag positions get 0*rden=0, fixed below)
            nc.vector.tensor_mul(stag5[:, :, :, 0:8], val4, bc(rden3))
            # action diagonal = relu(remain)/denom at column 17*i
            nc.gpsimd.tensor_copy(stag3[:, :, 0:121:17], t_diag3)

            fg = pair_pool.tile([128, BLK * 64], F32, tag="fg")
            fg4 = fg[:].rearrange("p (k a b) -> p k a b", a=N, b=N)
            nc.vector.tensor_mul(fg4, val4, bc(s3))

            t1 = pair_pool.tile([128, BLK * 64], F32, tag="t1")
            t14 = t1[:].rearrange("p (k a b) -> p k a b", a=N, b=N)
            nc.vector.tensor_sub(t14, queue4, raw4)

            # fq = relu(t1) on Pool; A = fq - fg
            a_t = tail_pool.tile([128, BLK * 64], F32, tag="a_t")
            a4 = a_t[:].rearrange("p (k a b) -> p k a b", a=N, b=N)
            ft = tail_pool.tile([128, BLK * 64], F32, tag="ft")
            nc.scalar.activation(ft[:], t1[:], ACTF.Relu)
            nc.vector.tensor_sub(a_t[:], ft[:], fg[:])

            arr = node_pool.tile([128, BLK * N], F32, tag="arr")
            arr3 = arr[:].rearrange("p (k j) -> p k j", j=N)
            raw_perm = raw[:].rearrange("p (k i j) -> p k j i", i=N, j=N)
            nc.vector.tensor_reduce(arr3, raw_perm, axis=mybir.AxisListType.X,
                                    op=ALU.add)

            z1 = node_pool.tile([128, BLK * N], F32, tag="z1")
            nc.gpsimd.tensor_sub(z1[:], arr[:], dep[:])
            z2 = node_pool.tile([128, BLK * N], F32, tag="z2")
            z23 = z2[:].rearrange("p (k i) -> p k i", i=N)
            nc.gpsimd.tensor_add(z23, veh3, m3)
            fv = node_pool.tile([128, BLK * N], F32, tag="fv")
            nc.gpsimd.tensor_add(fv[:], z1[:], z2[:])

            ints = node_pool.tile([128, BLK * N], F32, tag="ints")
            nc.gpsimd.tensor_mul(ints[:], s_t[:], rs[:])
            t5 = node_pool.tile([128, BLK * N], F32, tag="t5")
            nc.gpsimd.tensor_sub(t5[:], fv[:], ints[:])
            r2b = node_pool.tile([128, BLK * N], F32, tag="r2b")
            r2b3 = r2b[:].rearrange("p (k i) -> p k i", i=N)
            nc.scalar.activation(r2b[:], t5[:], ACTF.Relu, bias=0.0,
                                 scale=1.0 / (N - 1))

            # intention = -A + r2b_bcast
            intn = tail_pool.tile([128, BLK * 64], F32, tag="intn")
            nc.vector.tensor_sub(
                intn[:].rearrange("p (k a b) -> p k a b", a=N, b=N),
                bc(r2b3), a4,
            )

            # price = relu(0.4 - 0.25*relu(z)) + 0.6 on the ACT engine
            u_t = tail_pool.tile([128, BLK * 64], F32, tag="u_t")
            nc.scalar.activation(u_t[:], intn[:], ACTF.Relu)
            v_t = tail_pool.tile([128, BLK * 64], F32, tag="v_t")
            nc.scalar.activation(
                v_t[:], u_t[:], ACTF.Relu, bias=c04_t[:, 0:1], scale=-0.25
            )
            nc.scalar.activation(
                stag5[:, :, :, 8:16],
                v_t[:].rearrange("p (k a b) -> p k a b", a=N, b=N),
                ACTF.Copy, bias=0.6,
            )

            half = BLK // 2
            nc.sync.dma_start(out_r[b][:, 0:half, :], stag3[:, 0:half, :])
            nc.sync.dma_start(out_r[b][:, half:BLK, :], stag3[:, half:BLK, :])


def _build(bench_reps=None, stages='full', passes=1):
    nc = bacc.Bacc(
        "TRN2", target_bir_lowering=False, debug=False,
        enable_asserts=False, num_devices=NCORES,
    )
    xf = nc.dram_tensor("xf", [EPC, FI], FP16, kind="ExternalInput").ap()
    dmat_d = nc.dram_tensor("dmat", [FI, 128], FP16, kind="ExternalInput").ap()
    iden_d = nc.dram_tensor("iden", [128, 128], FP16, kind="ExternalInput").ap()
    outc = nc.dram_tensor("outc", [EPC, 2 * N * N], FP16, kind="ExternalOutput").ap()
    with tile.TileContext(nc) as tc:
        _kernel_body(tc, outc, xf, dmat_d, iden_d,
                     bench_reps=bench_reps, stages=stages, passes=passes)
    nc.compile()
    return nc


def _host_consts(W0, b0, W1, b1, dp, qp):
    n = np.arange(N)
    A0 = np.zeros((N, F), np.float64)
    A0[n, n] += W0[:, 0]
    for i in range(MINI):
        A0[n, N + N * n + i] += W0[:, 1 + i]
    for j in range(N):
        A0[n, 24 + N * n + j] += W0[:, 3 + j]
        A0[n, 24 + N * j + n] += W0[:, 11 + j]
    A1 = W1 @ A0
    c1 = W1 @ b0 + b1
    DM = dp[:, :, None] * (A1[:, None, :] - A1[None, :, :])
    dconst = (dp * (c1[:, None] - c1[None, :])).reshape(64)
    dmat = np.zeros((FI, 128), np.float64)
    dmat[:F, 0:64] = DM.reshape(64, F).T
    dmat[F, 0:64] = dconst                  # bias row, driven by ones column
    qpf = qp.astype(np.float64).copy()
    np.fill_diagonal(qpf, 0.0)
    for i in range(N):
        for j in range(N):
            dmat[24 + N * i + j, 64 + N * i + j] = qpf[i, j]
    iden = np.eye(128, dtype=np.float16)
    return dmat.astype(np.float16), iden


def kernel(x, W0, b0, W1, b1, distribute_param, queue_param, _trace=False):
    x = np.asarray(x, np.float32)
    W0 = np.asarray(W0, np.float64)
    b0 = np.asarray(b0, np.float64)
    W1 = np.asarray(W1, np.float64)
    b1 = np.asarray(b1, np.float64)
    dp = np.asarray(distribute_param, np.float64)
    qp = np.asarray(queue_param, np.float64)

    if "nc" not in _CACHE:
        _CACHE["nc"] = _build()
    nc = _CACHE["nc"]

    dmat, iden = _host_consts(W0, b0, W1, b1, dp, qp)
    xi = np.empty((EP, FI), np.float16)
    xi[:, :F] = x
    xi[:, F] = 1.0
    x8 = xi.reshape(NCORES, EPC, FI)
    in_maps = [
        {"xf": np.ascontiguousarray(x8[c]), "dmat": dmat, "iden": iden}
        for c in range(NCORES)
    ]
    res = run_bass_kernel_spmd(
        nc, in_maps, core_ids=list(range(NCORES)), trace=_trace
    )
    out = np.concatenate([res.results[c]["outc"] for c in range(NCORES)], axis=0)
    if _trace:
        _CACHE["last_results"] = res
    return out.astype(np.float32)



# revision 4
# speedup vs baseline: 2.3540x; 2.3540x over previous
"""Trainium2 Bass kernel for nn_ActionNetwork (gnn_message_passing).

Strategy (pure data parallel over the episode axis, 8 cores), v2:
  - fp16 everywhere in the tail: DVE tensor_tensor runs 2x for 16-bit
    step-1 operands and tensor_scalar runs 4x, so the elementwise tail
    (the v1 bottleneck: DVE 85% / ACT 83% busy) roughly halves.
  - One PE matmul per 128 episodes emits [diff | qq] (qq = queue *
    queue_param via one-hot columns).  qq >= 0 always, so a single ACT
    relu over both halves yields [relu(diff) | qq] in fp16 and
    val = va + qq is one 2x DVE add (no separate qq path).
  - Algebraic restructuring of the tail (exact, not approximate):
      rv + s = 1          -> fg = val - raw
      ints = rs - dep     -> t5 = (veh - rs) + m + arr   (dep cancels)
      intn = val + r2 - max(queue, raw)
      price = clip(1 - intn/4, 0.6, 1.0)   (two dual-op tensor_scalars)
    This removes the dep/ints/fv node chain, the fg/ft/a pair chain and
    two ACT relus of v1.
  - Reductions (rs over j, arr over i) as 3-level pairwise add trees:
    tensor_reduce is 1x-only on DVE while tree adds run 2x in fp16.
  - Work is balanced across DVE / ACT / GPSIMD: broadcasts of rden/r2
    on ACT (1x but a third stream), raw-mul + arr-l1 + m/t5 chain +
    stag diagonal on GPSIMD, everything else on DVE.
"""

import numpy as np

import concourse.bass as bass
import concourse.tile as tile
from concourse import bacc, mybir
from concourse.bass_utils import run_bass_kernel_spmd

F32 = mybir.dt.float32
FP16 = mybir.dt.float16
ALU = mybir.AluOpType
ACTF = mybir.ActivationFunctionType

N = 8
MINI = 2
EP = 131072
F = 88
FI = 89
NCORES = 8
EPC = EP // NCORES          # 16384 episodes per core
BLK = 16                    # 128-episode blocks per btile
BT = 128 * BLK              # 2048 episodes per btile
NBT = EPC // BT             # 8 btiles per core

_CACHE = {}


def _kernel_body(tc, outc, xf, dmat_d, iden_d, bench_reps=None, passes=1):
    nc = tc.nc
    from contextlib import ExitStack
    ctx = ExitStack()
    with ctx:
        ctx.enter_context(
            nc.allow_low_precision(reason="2e-2 output tolerance; fp16 tail")
        )
        const_pool = ctx.enter_context(tc.tile_pool(name="const", bufs=1))
        xin_pool = ctx.enter_context(tc.tile_pool(name="xin", bufs=3))
        xt_pool = ctx.enter_context(tc.tile_pool(name="xt", bufs=3))
        pair_pool = ctx.enter_context(tc.tile_pool(name="pair", bufs=2))
        tree_pool = ctx.enter_context(tc.tile_pool(name="tree", bufs=2))
        node_pool = ctx.enter_context(tc.tile_pool(name="node", bufs=2))
        bc_pool = ctx.enter_context(tc.tile_pool(name="bc", bufs=2))
        stag_pool = ctx.enter_context(tc.tile_pool(name="stag", bufs=3))
        ps_xt = ctx.enter_context(tc.tile_pool(name="ps_xt", bufs=2, space="PSUM"))
        ps_dq = ctx.enter_context(tc.tile_pool(name="ps_dq", bufs=3, space="PSUM"))

        iden_t = const_pool.tile([128, 128], FP16, tag="iden")
        nc.scalar.dma_start(iden_t[:], iden_d)
        dm_t = const_pool.tile([FI, 128], FP16, tag="dmat")
        nc.scalar.dma_start(dm_t[:], dmat_d)

        # episode -> (partition, chunk) mapping: partition p owns the 16
        # consecutive episodes [2048*bb + 16p, +16); chunk k picks the k-th.
        xf_r = xf.rearrange("(bb p k) f -> bb p k f", p=128, k=BLK)
        out_r = outc.rearrange("(bb p k) o -> bb p k o", p=128, k=BLK)

        def bcj(node_ap3):
            # (128, BLK, N) node tensor -> broadcast over trailing pair dim
            return node_ap3.unsqueeze(3).broadcast_to((128, BLK, N, N))

        if bench_reps is not None:
            loop_cm = tc.For_i(
                0, bench_reps, 1,
                hint_engines=(mybir.EngineType.PE, mybir.EngineType.DVE,
                              mybir.EngineType.Activation),
            )
            ctx.enter_context(loop_cm)

        for b in range(NBT * passes):
            b = b % NBT
            xin = xin_pool.tile([128, BLK * FI], FP16, tag="xin")
            xin3 = xin[:].rearrange("p (k f) -> p k f", f=FI)
            if b == 0:
                # fine-grained first load so the PE front-end starts early
                for c in range(BLK // 4):
                    nc.sync.dma_start(
                        xin3[:, 4 * c:4 * c + 4, :], xf_r[b][:, 4 * c:4 * c + 4, :]
                    )
            else:
                nc.sync.dma_start(xin3, xf_r[b])
            veh3 = xin3[:, :, 0:N]
            queue3 = xin3[:, :, 24:88]

            # front-end: per 128-episode chunk transpose + one [89->128]
            # matmul against [dmat | qmat]; relu over the full [diff | qq]
            # block (qq >= 0, relu is a no-op there) -> fp16 [va | qq]
            vaqq = pair_pool.tile([128, BLK * 128], FP16, tag="vaqq")
            vaqq3 = vaqq[:].rearrange("p (k c) -> p k c", c=128)
            xtA = ps_xt.tile([FI, BLK * 128], FP16, tag="xtA")
            for k in range(BLK):
                nc.tensor.transpose(
                    xtA[:, 128 * k:128 * k + 128], xin3[:, k, :], iden_t[:]
                )
            xt_c = xt_pool.tile([FI, BLK * 128], FP16, tag="xt")
            nc.vector.tensor_copy(xt_c[:], xtA[:])
            for c in range(BLK // 4):
                dq = ps_dq.tile([128, 512], F32, tag="dq")
                for kk in range(4):
                    k = 4 * c + kk
                    nc.tensor.matmul(dq[:, 128 * kk:128 * kk + 128],
                                     xt_c[:, 128 * k:128 * k + 128],
                                     dm_t[:],
                                     start=True, stop=True)
                nc.scalar.activation(
                    vaqq3[:, 4 * c:4 * c + 4, :], dq[:].rearrange(
                        "p (k c) -> p k c", c=128),
                    ACTF.Relu,
                )

            va3 = vaqq3[:, :, 0:64]
            qq3 = vaqq3[:, :, 64:128]

            # val = relu(diff) + qq  (both fp16 SBUF -> 2x)
            val = pair_pool.tile([128, BLK * 64], FP16, tag="val")
            val3 = val[:].rearrange("p (k d) -> p k d", d=64)
            val4 = val[:].rearrange("p (k a b) -> p k a b", a=N, b=N)
            nc.vector.tensor_add(val3, va3, qq3)

            # rs_i = sum_j val_ij via a pairwise tree (2x for l1/l2)
            rst1 = tree_pool.tile([128, BLK * 32], FP16, tag="rst1")
            rst14 = rst1[:].rearrange("p (k a h) -> p k a h", a=N, h=4)
            nc.vector.tensor_add(rst14, val4[:, :, :, 0:4], val4[:, :, :, 4:8])
            rst2 = tree_pool.tile([128, BLK * 16], FP16, tag="rst2")
            rst24 = rst2[:].rearrange("p (k a h) -> p k a h", a=N, h=2)
            nc.vector.tensor_add(rst24, rst14[:, :, :, 0:2], rst14[:, :, :, 2:4])
            rs = node_pool.tile([128, BLK * N], FP16, tag="rs")
            rs3 = rs[:].rearrange("p (k i) -> p k i", i=N)
            nc.vector.tensor_add(rs3, rst24[:, :, :, 0], rst24[:, :, :, 1])

            # node chain: denom = max(veh, rs); rden = 1/denom; rv = veh*rden
            # t_diag = 1 - rs*rden; rem = veh - rs
            denom = node_pool.tile([128, BLK * N], FP16, tag="denom")
            denom3 = denom[:].rearrange("p (k i) -> p k i", i=N)
            nc.vector.tensor_tensor(denom3, veh3, rs3, op=ALU.max)
            rden = node_pool.tile([128, BLK * N], FP16, tag="rden")
            rden3 = rden[:].rearrange("p (k i) -> p k i", i=N)
            nc.vector.reciprocal(rden[:], denom[:])
            rv = node_pool.tile([128, BLK * N], FP16, tag="rv")
            rv3 = rv[:].rearrange("p (k i) -> p k i", i=N)
            nc.vector.tensor_mul(rv3, veh3, rden3)
            g_t = node_pool.tile([128, BLK * N], FP16, tag="g_t")
            nc.vector.tensor_mul(g_t[:], rs[:], rden[:])
            t_diag = node_pool.tile([128, BLK * N], FP16, tag="t_diag")
            t_diag3 = t_diag[:].rearrange("p (k i) -> p k i", i=N)
            nc.vector.tensor_scalar(t_diag[:], g_t[:], -1.0, 1.0,
                                    op0=ALU.mult, op1=ALU.add)
            rem = node_pool.tile([128, BLK * N], FP16, tag="rem")
            rem3 = rem[:].rearrange("p (k i) -> p k i", i=N)
            nc.vector.tensor_sub(rem3, veh3, rs3)

            # raw = val * rv_i  (GPSIMD, rv broadcast in-AP)
            raw = pair_pool.tile([128, BLK * 64], FP16, tag="raw")
            raw3 = raw[:].rearrange("p (k d) -> p k d", d=64)
            raw4 = raw[:].rearrange("p (k a b) -> p k a b", a=N, b=N)
            nc.gpsimd.tensor_mul(raw4, val4, bcj(rv3))

            # action = val * rden_i -> stag action slots (diag fixed below)
            rdbc = bc_pool.tile([128, BLK * 64], FP16, tag="rdbc")
            rdbc4 = rdbc[:].rearrange("p (k a b) -> p k a b", a=N, b=N)
            nc.scalar.activation(rdbc4, bcj(rden3), ACTF.Copy)
            stag = stag_pool.tile([128, BLK * 128], FP16, tag="stag")
            stag5 = stag[:].rearrange("p (k i c) -> p k i c", i=N, c=2 * N)
            stag3 = stag[:].rearrange("p (k d) -> p k d", d=2 * N * N)
            nc.vector.tensor_mul(stag5[:, :, :, 0:8], val4, rdbc4)
            nc.gpsimd.tensor_copy(stag3[:, :, 0:121:17], t_diag3)

            # arr_j = sum_i raw_ij tree: l1 on GPSIMD, l2+l3 on DVE
            rawt1 = tree_pool.tile([128, BLK * 32], FP16, tag="rawt1")
            rawt14 = rawt1[:].rearrange("p (k h b) -> p k h b", h=4, b=N)
            nc.gpsimd.tensor_add(rawt14, raw4[:, :, 0:4, :], raw4[:, :, 4:8, :])
            rawt2 = tree_pool.tile([128, BLK * 16], FP16, tag="rawt2")
            rawt24 = rawt2[:].rearrange("p (k h b) -> p k h b", h=2, b=N)
            nc.vector.tensor_add(rawt24, rawt14[:, :, 0:2, :], rawt14[:, :, 2:4, :])
            arr = node_pool.tile([128, BLK * N], FP16, tag="arr")
            arr3 = arr[:].rearrange("p (k j) -> p k j", j=N)
            nc.vector.tensor_add(arr3, rawt24[:, :, 0, :], rawt24[:, :, 1, :])

            # t5 = (veh - rs) + (m + arr); r2 = relu(t5)/7
            m_t = node_pool.tile([128, BLK * N], FP16, tag="m_t")
            m3 = m_t[:].rearrange("p (k i) -> p k i", i=N)
            nc.gpsimd.tensor_add(m3, xin3[:, :, 8:24:2], xin3[:, :, 9:24:2])
            marr = node_pool.tile([128, BLK * N], FP16, tag="marr")
            nc.gpsimd.tensor_add(marr[:], m_t[:], arr[:])
            t5 = node_pool.tile([128, BLK * N], FP16, tag="t5")
            nc.gpsimd.tensor_add(t5[:], rem[:], marr[:])
            r2 = node_pool.tile([128, BLK * N], FP16, tag="r2")
            r23 = r2[:].rearrange("p (k i) -> p k i", i=N)
            nc.vector.tensor_scalar(r2[:], t5[:], 0.0, 1.0 / (N - 1),
                                    op0=ALU.max, op1=ALU.mult)
            r2bc = bc_pool.tile([128, BLK * 64], FP16, tag="r2bc")
            r2bc3 = r2bc[:].rearrange("p (k d) -> p k d", d=64)
            nc.scalar.activation(
                r2bc[:].rearrange("p (k a b) -> p k a b", a=N, b=N),
                bcj(r23), ACTF.Copy,
            )

            # intn = (val + r2) - max(queue, raw)
            mx = pair_pool.tile([128, BLK * 64], FP16, tag="mx")
            mx3 = mx[:].rearrange("p (k d) -> p k d", d=64)
            nc.vector.tensor_tensor(mx3, queue3, raw3, op=ALU.max)
            s1 = pair_pool.tile([128, BLK * 64], FP16, tag="s1")
            nc.vector.tensor_add(s1[:], val[:], r2bc[:])
            intn = pair_pool.tile([128, BLK * 64], FP16, tag="intn")
            nc.vector.tensor_sub(intn[:], s1[:], mx[:])

            # price = clip(1 - intn/4, 0.6, 1.0)
            p1 = pair_pool.tile([128, BLK * 64], FP16, tag="p1")
            nc.vector.tensor_scalar(p1[:], intn[:], -0.25, 1.0,
                                    op0=ALU.mult, op1=ALU.add)
            nc.vector.tensor_scalar(
                stag5[:, :, :, 8:16],
                p1[:].rearrange("p (k a b) -> p k a b", a=N, b=N),
                0.6, 1.0, op0=ALU.max, op1=ALU.min,
            )

            half = BLK // 2
            nc.sync.dma_start(out_r[b][:, 0:half, :], stag3[:, 0:half, :])
            nc.sync.dma_start(out_r[b][:, half:BLK, :], stag3[:, half:BLK, :])


def _build(bench_reps=None, stages='full', passes=1):
    nc = bacc.Bacc(
        "TRN2", target_bir_lowering=False, debug=False,
        enable_asserts=False, num_devices=NCORES,
    )
    xf = nc.dram_tensor("xf", [EPC, FI], FP16, kind="ExternalInput").ap()
    dmat_d = nc.dram_tensor("dmat", [FI, 128], FP16, kind="ExternalInput").ap()
    iden_d = nc.dram_tensor("iden", [128, 128], FP16, kind="ExternalInput").ap()
    outc = nc.dram_tensor("outc", [EPC, 2 * N * N], FP16, kind="ExternalOutput").ap()
    with tile.TileContext(nc) as tc:
        _kernel_body(tc, outc, xf, dmat_d, iden_d,
                     bench_reps=bench_reps, passes=passes)
    nc.compile()
    return nc


def _host_consts(W0, b0, W1, b1, dp, qp):
    n = np.arange(N)
    A0 = np.zeros((N, F), np.float64)
    A0[n, n] += W0[:, 0]
    for i in range(MINI):
        A0[n, N + N * n + i] += W0[:, 1 + i]
    for j in range(N):
        A0[n, 24 + N * n + j] += W0[:, 3 + j]
        A0[n, 24 + N * j + n] += W0[:, 11 + j]
    A1 = W1 @ A0
    c1 = W1 @ b0 + b1
    DM = dp[:, :, None] * (A1[:, None, :] - A1[None, :, :])
    dconst = (dp * (c1[:, None] - c1[None, :])).reshape(64)
    dmat = np.zeros((FI, 128), np.float64)
    dmat[:F, 0:64] = DM.reshape(64, F).T
    dmat[F, 0:64] = dconst                  # bias row, driven by ones column
    qpf = qp.astype(np.float64).copy()
    np.fill_diagonal(qpf, 0.0)
    for i in range(N):
        for j in range(N):
            dmat[24 + N * i + j, 64 + N * i + j] = qpf[i, j]
    iden = np.eye(128, dtype=np.float16)
    return dmat.astype(np.float16), iden


def kernel(x, W0, b0, W1, b1, distribute_param, queue_param, _trace=False):
    x = np.asarray(x, np.float32)
    W0 = np.asarray(W0, np.float64)
    b0 = np.asarray(b0, np.float64)
    W1 = np.asarray(W1, np.float64)
    b1 = np.asarray(b1, np.float64)
    dp = np.asarray(distribute_param, np.float64)
    qp = np.asarray(queue_param, np.float64)

    if "nc" not in _CACHE:
        _CACHE["nc"] = _build()
    nc = _CACHE["nc"]

    dmat, iden = _host_consts(W0, b0, W1, b1, dp, qp)
    xi = np.empty((EP, FI), np.float16)
    xi[:, :F] = x
    xi[:, F] = 1.0
    x8 = xi.reshape(NCORES, EPC, FI)
    in_maps = [
        {"xf": np.ascontiguousarray(x8[c]), "dmat": dmat, "iden": iden}
        for c in range(NCORES)
    ]
    res = run_bass_kernel_spmd(
        nc, in_maps, core_ids=list(range(NCORES)), trace=_trace
    )
    out = np.concatenate([res.results[c]["outc"] for c in range(NCORES)], axis=0)
    if _trace:
        _CACHE["last_results"] = res
    return out.astype(np.float32)
